# revision 36
# baseline (speedup 1.0000x reference)
"""Trainium2 Bass kernel for nn_AttentionProjector (8-core SPMD), v5.

Math: out = softmax(q @ (x@Wk.T).T) @ (x@Wv.T + Wv_b)
Rewritten (FLOP reduction):
    scores = (q @ Wk) @ x.T      (Wk_b cancels in softmax)
    out    = (softmax(scores) @ x) @ Wv.T + Wv_b

Structure (8 cores), v5 notes:
  - ALL collectives use the single 8-rank replica group. (v4 tried a
    pair-group warm-up; the extra communicator halved the bandwidth of
    every later collective: AG-q 18.6->32us, AR halves 30/37us.)
  - warm-up AllGather at t=0 absorbs the ~80us ncfw first-collective
    cold start while phase 1 + input DMAs run.
  - phase 1: q'T slice via Wk[:,ds_j] -> AllGather q'T (ship on sync
    ring ahead of the xT descriptors, doorbell on gpsimd).
  - phase 2: scores = q'T.T @ xT_j (f16). Local row max -> AllReduce-MAX
    (CCE computes the global max in the DMA path; no readback chain).
    m ship + M readback ride HWDGE rings (sync/scalar) - the v4 SWDGE
    path added ~8us latency each way.
  - factor exp(m_j - M) and s' = S_j*f on the SCALAR queue only; facb
    broadcast via PE between h0 and h1 so nothing head-of-line blocks
    the phase-3 PSUM casts (the v3 25us stall).
  - phase 3: uT = x.T @ p.T, xb SBUF-resident, two d-tiles per PSUM
    bank. h0 casts plain + rescale, shipped immediately; h1 casts fuse
    the rescale (psu*facb -> bf16). Single AllReduce for u + s'.
  - phase 4: after a warm bridge, 4 ctx chunks read back + consumed in
    order; out = (ctxT/S).T @ WvT + Wv_b.

Precision: score path f16; values path bf16 (measured 5.5e-3).
"""

import numpy as np

L = 256          # query rows
D = 4096         # d_in == d_out
N = 8192         # tokens
NCORES = 8
NS = N // NCORES     # 1024 tokens per core
DS = D // NCORES     # 512 dout per core

LT = L // 128        # 2 l-tiles
DT = D // 128        # 32 d-tiles
NT = NS // 128       # 8 local n-tiles
HT = DT // 2         # 16 d-tiles per u half

_MAX_WAITS = 1


def _split_waits(nc, mybir, bass_rust):
    """Walrus in this container allows only one sync-wait per instruction;
    move excess waits onto preceding same-engine no-ops."""
    for bb in nc.main_func.blocks:
        new_list = []
        for ins in bb.instructions:
            si = ins.sync_info
            waits = list(si.on_wait) if si is not None else []
            if len(waits) > _MAX_WAITS:
                for i in range(_MAX_WAITS, len(waits), _MAX_WAITS):
                    nop = mybir.InstNoOp(name=f"{ins.name}-wsplit{i}", ins=[], outs=[])
                    nop.engine = ins.engine
                    nop.sync_info = bass_rust.SyncInfo(
                        on_wait=waits[i:i + _MAX_WAITS], on_update=[])
                    new_list.append(nop)
                ins.sync_info = bass_rust.SyncInfo(
                    on_wait=waits[:_MAX_WAITS], on_update=si.on_update)
            new_list.append(ins)
        bb.instructions[:] = new_list


_NC = None


def _build(split_waits=True):
    global _NC
    if _NC is not None and split_waits:
        return _NC
    import bass_rust
    import concourse.bass as bass
    import concourse.mybir as mybir
    import concourse.tile as tile
    from concourse.masks import make_identity
    from contextlib import ExitStack

    f32 = mybir.dt.float32
    bf16 = mybir.dt.bfloat16
    AF = mybir.ActivationFunctionType
    AX = mybir.AxisListType
    ALU = mybir.AluOpType
    RG = [list(range(NCORES))]

    f16 = mybir.dt.float16
    nc = bass.Bass()

    # per-core external I/O
    t_qts = nc.dram_tensor("qTs", [D, L], f16, kind="ExternalInput")
    t_wk = nc.dram_tensor("wk", [D, DS], f16, kind="ExternalInput")
    t_xt = nc.dram_tensor("xT", [D, NS], f16, kind="ExternalInput")
    t_xb = nc.dram_tensor("xb", [NS, D], bf16, kind="ExternalInput")
    t_wvt = nc.dram_tensor("wvT", [D, DS], bf16, kind="ExternalInput")
    t_wvb = nc.dram_tensor("wvb", [1, DS], bf16, kind="ExternalInput")
    t_out = nc.dram_tensor("out", [L, DS], f32, kind="ExternalOutput")

    # collective bounce buffers (input Local, output Shared)
    warm_in = nc.dram_tensor("warm_in", [1, 128], f32)
    warm_out = nc.dram_tensor("warm_out", [NCORES, 128], f32, addr_space="Shared")
    ar_q_in = nc.dram_tensor("ar_q_in", [DS, L], f16)
    ar_q_out = nc.dram_tensor("ar_q_out", [D, L], f16, addr_space="Shared")
    ar_m_in = nc.dram_tensor("ar_m_in", [L, 1], f32)
    ar_m_out = nc.dram_tensor("ar_m_out", [L, 1], f32, addr_space="Shared")
    # u: 32 d-tile blocks + one s' block, single AllReduce payload
    ar_u_in = nc.dram_tensor("ar_u_in", [(DT + 1) * 128, L], bf16)
    ar_u_out = nc.dram_tensor("ar_u_out", [(DT + 1) * 128, L], bf16,
                              addr_space="Shared")

    # inputs are host-permuted to p-major so every DMA chunk reads long
    # contiguous per-partition lines (4-16KB instead of 1KB)
    qts_re = t_qts.ap().rearrange("(p kt) l -> p kt l", p=128)   # [128, 32, 256]
    wk_re = t_wk.ap().rearrange("(p kt) d -> p kt d", p=128)     # [128, 32, 512]
    xt_re = t_xt.ap().rearrange("(p dt) n -> p dt n", p=128)     # [128, 32, 1024]
    xb_re = t_xb.ap().rearrange("(p nt) d -> p nt d", p=128)     # [128, 8, 4096]
    wvt_re = t_wvt.ap().rearrange("(p dt) o -> p dt o", p=128)   # [128, 32, 512]
    # AG-q bounce: p-major WITHIN each core's block, so ship writes and the
    # per-block readback both move 2KB contiguous per-partition lines (the
    # (dt p) layout read 512B lines and cost ~13us before phase 2)
    arq_re = ar_q_in.ap().rearrange("(p dtl) l -> p dtl l", p=128)
    arqo_re = ar_q_out.ap().rearrange("(j p dtl) l -> p j dtl l", p=128, dtl=4)
    aru_re = ar_u_in.ap().rearrange("(t p) l -> p t l", p=128)
    aruo_re = ar_u_out.ap().rearrange("(t p) l -> p t l", p=128)

    with ExitStack() as ctx:
        tc = ctx.enter_context(tile.TileContext(nc))
        const = ctx.enter_context(tc.tile_pool(name="const", bufs=1))
        small = ctx.enter_context(tc.tile_pool(name="small", bufs=1))
        persist = ctx.enter_context(tc.tile_pool(name="persist", bufs=1))

        # ---- warm-up: tiny collective absorbs ncfw cold-start ---------------
        # 8-rank AllGather, same replica group as every other collective (a
        # second communicator halves collective bandwidth - v4 lesson), and
        # same kind as AG-q (an AllReduce-max warm-up made AG-q ~10us slower
        # in two separate experiments, v6/v8).
        nc.gpsimd.collective_compute(
            "AllGather", ALU.bypass, replica_groups=RG,
            ins=[warm_in.ap().opt()], outs=[warm_out.ap().opt()])

        ident_bf = const.tile([128, 128], bf16)
        make_identity(nc, ident_bf[:])
        ident_f = const.tile([128, 128], f32)
        make_identity(nc, ident_f[:])
        ones1 = const.tile([1, 128], f32)
        nc.vector.memset(ones1[:], 1.0)
        bias_sb = const.tile([128, DS], bf16)
        wvb_sb = const.tile([1, DS], bf16)
        nc.scalar.dma_start(wvb_sb[:], t_wvb.ap())
        ones1b = const.tile([1, 128], bf16)
        nc.vector.memset(ones1b[:], 1.0)
        s_blk = const.tile([128, L], bf16)       # s' payload block (zeros + 2 cols)
        nc.vector.memset(s_blk[:], 0.0)
        # prewarm the scalar Exp activation table (1.3us off the softmax path)
        dum = const.tile([1, 2], f32)
        nc.vector.memset(dum[:], 0.0)
        nc.scalar.activation(dum[:], dum[:], AF.Exp)

        # PE clock-gate warm-up during the first input DMAs
        with tc.tile_pool(name="warmps", bufs=1, space="PSUM") as warmps:
            wps = warmps.tile([128, 128], f32)
            for i in range(28):
                nc.tensor.matmul(wps[:], ident_bf[:], ident_bf[:],
                                 start=(i == 0), stop=(i == 27))
            # bias broadcast [1,DS] -> [128,DS] via rank-1 matmul
            bps = warmps.tile([128, DS], f32)
            nc.tensor.matmul(bps[:], ones1b[:], wvb_sb[:], start=True, stop=True)
            nc.vector.tensor_copy(bias_sb[:], bps[:])

        # persistent across phases
        pT = persist.tile([128, NT, L], bf16)        # p.T (0.5MB)
        xb_all = persist.tile([128, NT, D], bf16)    # x_j resident (8MB)

        wv_pool = ctx.enter_context(tc.tile_pool(name="ph4w", bufs=2))
        WVCH = 8                     # d-tiles per wv chunk (1MB)

        def wv_load(c):
            wv_c = wv_pool.tile([128, WVCH, DS], bf16, name="wv_c")
            nc.scalar.dma_start(wv_c[:], wvt_re[:, c * WVCH:(c + 1) * WVCH, :])
            return wv_c

        # ---------------- phase 1: q'T partial = Wk[:,ds_j].T @ q.T ----------
        with tc.tile_pool(name="ph1q", bufs=2) as ph1q, \
             tc.tile_pool(name="ph1wk", bufs=4) as ph1wk, \
             tc.tile_pool(name="ph1ps", bufs=1, space="PSUM") as ph1ps:
            def qts_load(qc):
                qts_c = ph1q.tile([128, 8, L], f16, name="qts_c")
                nc.sync.dma_start(qts_c[:], qts_re[:, qc * 8:(qc + 1) * 8, :])
                return qts_c
            qts_cs = [qts_load(0)]
            qpT_loc = ph1q.tile([128, 4, L], f16, name="qpT_loc")
            ps4 = [ph1ps.tile([128, L], f32, name=f"ph1ps{i}") for i in range(4)]
            KCH = 4                                  # k-tiles per wk chunk (1MB)
            for kc in range(DT // KCH):
                wk_c = ph1wk.tile([128, KCH, DS], f16, name="wk_c")
                nc.sync.dma_start(wk_c[:], wk_re[:, kc * KCH:(kc + 1) * KCH, :])
                if kc % 2 == 0 and kc // 2 + 1 < 4:
                    qts_cs.append(qts_load(kc // 2 + 1))
                for i in range(KCH):
                    kt = kc * KCH + i
                    for dtl in range(4):
                        nc.tensor.matmul(
                            ps4[dtl][:], wk_c[:, i, dtl * 128:(dtl + 1) * 128],
                            qts_cs[kt // 8][:, kt % 8, :],
                            start=(kt == 0), stop=(kt == DT - 1))
            for dtl in range(4):
                nc.vector.tensor_copy(qpT_loc[:, dtl, :], ps4[dtl][:])
            # ship on the sync ring (ahead of the xT stream descriptors)
            nc.sync.dma_start(arq_re, qpT_loc[:])
            nc.gpsimd.collective_compute(
                "AllGather", ALU.bypass, replica_groups=RG,
                ins=[ar_q_in.ap().opt()], outs=[ar_q_out.ap().opt()])

        # q'T readback: one chunk per core block on the scalar ring; phase 2
        # starts as soon as block 0 lands
        qpT = persist.tile([128, DT, L], f16, name="qpT")
        for j in range(NCORES):
            nc.scalar.dma_start(qpT[:, 4 * j:4 * j + 4, :],
                                arqo_re[:, j, :, :])

        # ---------------- phase 2: scores[l, n_j] ----------------------------
        XCH = 8                      # d-tiles per xT chunk (2MB)
        m_both = small.tile([128, 2], f32, name="m_both")
        s_both = small.tile([128, 2], f32, name="s_both")
        p_sb = [persist.tile([128, NS], bf16, name=f'p_sb{i}') for i in range(LT)]
        with tc.tile_pool(name="ph2xt", bufs=3) as xt_pool, \
             tc.tile_pool(name="ph2sc", bufs=1, space="PSUM") as scps_pool:
            score_ps = [[scps_pool.tile([128, 512], f32, name=f'score{i}_{k}')
                         for k in range(2)] for i in range(LT)]
            for c in range(DT // XCH):
                xt_c = xt_pool.tile([128, XCH, NS], f16, name="xt_c")
                nc.sync.dma_start(xt_c[:], xt_re[:, c * XCH:(c + 1) * XCH, :])
                for i in range(XCH):
                    dt = c * XCH + i
                    for lt in range(LT):
                        for nch in range(2):
                            nc.tensor.matmul(
                                score_ps[lt][nch][:],
                                qpT[:, dt, lt * 128:(lt + 1) * 128],
                                xt_c[:, i, nch * 512:(nch + 1) * 512],
                                start=(dt == 0), stop=(dt == DT - 1))

            # xb load rides the sync ring in the post-xT window (v4 loaded it
            # at t=0 on scalar and starved phase 1's wk stream)
            for cb in range(4):
                nc.sync.dma_start(xb_all[:, 2 * cb:2 * cb + 2, :],
                                  xb_re[:, 2 * cb:2 * cb + 2, :])

            # local row max; ship m_j on the (idle) sync ring, then
            # AllReduce-MAX: the CCE computes the global row max in flight.
            negm = small.tile([128, 2], f32, name="negm")
            for lt in range(LT):
                mtmp = small.tile([128, 1], f32, name=f"mtmp{lt}")
                nc.vector.tensor_reduce(mtmp[:], score_ps[lt][0][:], axis=AX.X, op=ALU.max)
                nc.vector.tensor_reduce(m_both[:, lt:lt + 1], score_ps[lt][1][:],
                                        axis=AX.X, op=ALU.max)
                nc.vector.tensor_tensor(m_both[:, lt:lt + 1], m_both[:, lt:lt + 1],
                                        mtmp[:], ALU.max)
                nc.vector.tensor_scalar_mul(negm[:, lt:lt + 1],
                                            m_both[:, lt:lt + 1], -1.0)
                if lt == LT - 1:
                    nc.sync.dma_start(
                        ar_m_in.ap().rearrange("(lt p) o -> p (lt o)", p=128),
                        m_both[:])
                    nc.gpsimd.collective_compute(
                        "AllReduce", ALU.max, replica_groups=RG,
                        ins=[ar_m_in.ap().opt()], outs=[ar_m_out.ap().opt()])
                # exp for this lt starts as soon as ITS row max lands (the
                # serialized all-rowmax -> all-exp order cost ~2us)
                sp0 = small.tile([128, 1], f32, name=f"sp0_{lt}")
                nc.scalar.activation(p_sb[lt][:, 0:512], score_ps[lt][0][:],
                                     AF.Exp, bias=negm[:, lt:lt + 1], accum_out=sp0[:])
                nc.scalar.activation(p_sb[lt][:, 512:1024], score_ps[lt][1][:],
                                     AF.Exp, bias=negm[:, lt:lt + 1],
                                     accum_out=s_both[:, lt:lt + 1])
                nc.vector.tensor_tensor(s_both[:, lt:lt + 1], s_both[:, lt:lt + 1],
                                        sp0[:], ALU.add)

        # ---------------- transpose p -> pT [n, l] ---------------------------
        with tc.tile_pool(name="tp", bufs=2, space="PSUM") as tpps:
            for lt in range(LT):
                for nt in range(NT):
                    tp = tpps.tile([128, 128], bf16)
                    nc.tensor.transpose(
                        tp[:], p_sb[lt][:, nt * 128:(nt + 1) * 128], ident_bf[:])
                    nc.vector.tensor_copy(pT[:, nt, lt * 128:(lt + 1) * 128], tp[:])

        # ---- factor = exp(m_j - M): readback on scalar ring, math on SCALAR -
        M_sb = small.tile([128, 2], f32, name="M_sb")
        nc.scalar.dma_start(
            M_sb[:], ar_m_out.ap().rearrange("(lt p) o -> p (lt o)", p=128))
        fac2 = small.tile([128, 2], f32, name="fac2")
        for lt in range(LT):
            # fac = exp(-M + m_j)
            nc.scalar.activation(fac2[:, lt:lt + 1], M_sb[:, lt:lt + 1],
                                 AF.Exp, bias=m_both[:, lt:lt + 1], scale=-1.0)
            # s' = s_j * fac (cast to bf16 into the AR payload block)
            nc.scalar.activation(s_blk[:, lt:lt + 1], s_both[:, lt:lt + 1],
                                 AF.Copy, scale=fac2[:, lt:lt + 1])

        u_h = [persist.tile([128, HT, L], bf16, name=f"u_h{h}") for h in range(2)]
        ctx_c = [persist.tile([128, WVCH, L], bf16, name=f"ctx_c{c}")
                 for c in range(4)]
        s_ctx = persist.tile([128, L], bf16, name="s_ctx")
        facb2 = small.tile([128, 2, L], f32, name="facb2")

        # pre-claim the bridge's PSUM bank BEFORE ph3ps/facps take the other
        # 7, so the bridge never WAR-waits on a phase-3 bank release (that
        # wait cost 6.6us of PE idle at the h1->bridge boundary)
        warmps2 = ctx.enter_context(tc.tile_pool(name="warmps2", bufs=1,
                                                 space="PSUM"))
        wps2 = warmps2.tile([128, 128], f32)

        with tc.tile_pool(name="ph3ps", bufs=4, space="PSUM") as ph3ps:
            # ---- phase 3 h0: d-tiles 0..15, two per PSUM bank ---------------
            for dp in range(HT // 2):
                psu = ph3ps.tile([128, 2, L], f32)
                for k in range(2):
                    dt = 2 * dp + k
                    for nt in range(NT):
                        nc.tensor.matmul(
                            psu[:, k, :], xb_all[:, nt, dt * 128:(dt + 1) * 128],
                            pT[:, nt, :], start=(nt == 0), stop=(nt == NT - 1))
                nc.vector.tensor_copy(u_h[0][:, 2 * dp:2 * dp + 2, :], psu[:])

            # ---- factor broadcast via PE (tensor queue after h0 MMs; the
            # PSUM->SBUF copies ride the scalar queue) ------------------------
            with tc.tile_pool(name="facps", bufs=1, space="PSUM") as facps_pool:
                fb_ps = facps_pool.tile([128, L], f32, name="fb_ps")
                for lt in range(LT):
                    fac_ps = facps_pool.tile([1, 128], f32, name=f"fac_ps{lt}")
                    nc.tensor.transpose(fac_ps[:], fac2[:, lt:lt + 1], ident_f[:])
                    facr = small.tile([1, 128], f32, name=f"facr{lt}")
                    nc.scalar.activation(facr[:], fac_ps[:], AF.Copy)
                    nc.tensor.matmul(fb_ps[:, lt * 128:(lt + 1) * 128],
                                     ones1[:], facr[:], start=True, stop=True)
                # two copies of the broadcast so rescales handle a whole
                # 2-tile PSUM bank in one vector op
                for k in range(2):
                    nc.scalar.activation(facb2[:, k, :], fb_ps[:], AF.Copy)

            # rescale h0 (paired tiles) and ship its half of the AR payload
            for dp in range(HT // 2):
                nc.vector.tensor_tensor(u_h[0][:, 2 * dp:2 * dp + 2, :],
                                        u_h[0][:, 2 * dp:2 * dp + 2, :],
                                        facb2[:], ALU.mult)
            nc.sync.dma_start(aru_re[:, 0:HT, :], u_h[0][:])
            nc.sync.dma_start(aru_re[:, DT, :], s_blk[:])

            # ---- phase 3 h1: d-tiles 16..31, rescale fused into the cast ----
            for dp in range(HT // 2):
                psu = ph3ps.tile([128, 2, L], f32)
                for k in range(2):
                    dt = HT + 2 * dp + k
                    for nt in range(NT):
                        nc.tensor.matmul(
                            psu[:, k, :], xb_all[:, nt, dt * 128:(dt + 1) * 128],
                            pT[:, nt, :], start=(nt == 0), stop=(nt == NT - 1))
                nc.vector.tensor_tensor(u_h[1][:, 2 * dp:2 * dp + 2, :],
                                        psu[:], facb2[:], ALU.mult)
            nc.sync.dma_start(aru_re[:, HT:DT, :], u_h[1][:])
            nc.gpsimd.collective_compute(
                "AllReduce", ALU.add, replica_groups=RG,
                ins=[ar_u_in.ap().opt()], outs=[ar_u_out.ap().opt()])

        wv_cs = [wv_load(0), wv_load(1), wv_load(2), wv_load(3)]

        # warm bridge across the AR-u wait (no data deps): must span h1-end
        # (~184us) to ctx-readback-done (~253us), ~69us at ~83ns/MM warm
        for i in range(830):
            nc.tensor.matmul(wps2[:], ident_bf[:], ident_bf[:],
                             start=(i == 0), stop=(i == 829))

        # ctx readback (4 chunks, consumed in order by phase 4) + 1/S
        for c in range(4):
            nc.scalar.dma_start(ctx_c[c][:],
                                aruo_re[:, c * WVCH:(c + 1) * WVCH, :])
        nc.scalar.dma_start(s_ctx[:], aruo_re[:, DT, :])
        s_f = small.tile([128, 2], f32, name="s_f")
        nc.vector.tensor_copy(s_f[:], s_ctx[:, 0:2])
        rec2 = small.tile([128, 2], f32, name="rec2")
        nc.vector.reciprocal(rec2[:], s_f[:])

        # ---------------- phase 4: out = (ctxT/S).T @ WvT + Wv_b -------------
        with tc.tile_pool(name="ph4ps", bufs=1, space="PSUM") as ph4ps, \
             tc.tile_pool(name="ph4o", bufs=2) as out_pool:
            po = [ph4ps.tile([128, DS], f32, name=f'po{i}') for i in range(LT)]
            for c in range(4):
                for i in range(WVCH):
                    dt = c * WVCH + i
                    for lt in range(LT):
                        nc.tensor.matmul(
                            po[lt][:], ctx_c[c][:, i, lt * 128:(lt + 1) * 128],
                            wv_cs[c][:, i, :], start=(dt == 0), stop=(dt == DT - 1))
            for lt in range(LT):
                o_sb = out_pool.tile([128, DS], f32)
                nc.scalar.activation(o_sb[:], po[lt][:], AF.Copy,
                                     scale=rec2[:, lt:lt + 1])
                nc.vector.tensor_tensor(o_sb[:], o_sb[:], bias_sb[:], ALU.add)
                nc.scalar.dma_start(t_out[lt * 128:(lt + 1) * 128, :], o_sb[:])

    if split_waits:
        _split_waits(nc, mybir, bass_rust)
        _NC = nc
    return nc


last_results = None


def kernel(src_prompts, query, Wk_w, Wk_b, Wv_w, Wv_b):
    global last_results
    import ml_dtypes
    from concourse.bass_utils import run_bass_kernel_spmd

    nc = _build()

    x = np.ascontiguousarray(np.asarray(src_prompts, dtype=np.float32)[0])
    q = np.asarray(query, dtype=np.float32)
    wk = np.asarray(Wk_w, dtype=np.float32)
    wv = np.asarray(Wv_w, dtype=np.float32)
    wvb = np.asarray(Wv_b, dtype=np.float32)
    # Wk_b shifts every score row by a constant -> cancels in softmax.

    def pmajor(a, tiles):
        # [tiles*128, cols] row=t*128+p  ->  row=p*tiles+t (p-major), so each
        # SBUF partition's stream is contiguous in DRAM
        r, c = a.shape
        return np.ascontiguousarray(
            a.reshape(tiles, 128, c).transpose(1, 0, 2).reshape(r, c))

    qT = pmajor(np.ascontiguousarray(q.T).astype(np.float16), DT)
    in_maps = []
    for j in range(NCORES):
        ns, ds = slice(j * NS, (j + 1) * NS), slice(j * DS, (j + 1) * DS)
        xj = x[ns]
        in_maps.append({
            "qTs": qT,
            "wk": pmajor(wk[:, ds].astype(np.float16), DT),
            "xT": pmajor(np.ascontiguousarray(xj.T).astype(np.float16), DT),
            "xb": pmajor(xj.astype(ml_dtypes.bfloat16), NT),
            "wvT": pmajor(np.ascontiguousarray(wv[ds].T).astype(ml_dtypes.bfloat16), DT),
            "wvb": np.ascontiguousarray(wvb[ds][None, :].astype(ml_dtypes.bfloat16)),
        })

    res = run_bass_kernel_spmd(nc, in_maps, core_ids=list(range(NCORES)))
    last_results = res
    out = np.concatenate([res.results[j]["out"] for j in range(NCORES)], axis=1)
    return out[None, :, :]


# revision 41
# speedup vs baseline: 1.0181x; 1.0181x over previous
"""Trainium2 Bass kernel for nn_AttentionProjector (8-core SPMD), v5.

Math: out = softmax(q @ (x@Wk.T).T) @ (x@Wv.T + Wv_b)
Rewritten (FLOP reduction):
    scores = (q @ Wk) @ x.T      (Wk_b cancels in softmax)
    out    = (softmax(scores) @ x) @ Wv.T + Wv_b

Structure (8 cores), v5 notes:
  - ALL collectives use the single 8-rank replica group. (v4 tried a
    pair-group warm-up; the extra communicator halved the bandwidth of
    every later collective: AG-q 18.6->32us, AR halves 30/37us.)
  - warm-up AllGather at t=0 absorbs the ~80us ncfw first-collective
    cold start while phase 1 + input DMAs run.
  - phase 1: q'T slice via Wk[:,ds_j] -> AllGather q'T (ship on sync
    ring ahead of the xT descriptors, doorbell on gpsimd).
  - phase 2: scores = q'T.T @ xT_j (f16). Local row max -> AllReduce-MAX
    (CCE computes the global max in the DMA path; no readback chain).
    m ship + M readback ride HWDGE rings (sync/scalar) - the v4 SWDGE
    path added ~8us latency each way.
  - factor exp(m_j - M) and s' = S_j*f on the SCALAR queue only; facb
    broadcast via PE between h0 and h1 so nothing head-of-line blocks
    the phase-3 PSUM casts (the v3 25us stall).
  - phase 3: uT = x.T @ p.T, xb SBUF-resident, two d-tiles per PSUM
    bank. h0 casts plain + rescale, shipped immediately; h1 casts fuse
    the rescale (psu*facb -> bf16). Single AllReduce for u + s'.
  - phase 4: after a warm bridge, 4 ctx chunks read back + consumed in
    order; out = (ctxT/S).T @ WvT + Wv_b.

Precision: score path f16; values path bf16 (measured 5.5e-3).
"""

import numpy as np

L = 256          # query rows
D = 4096         # d_in == d_out
N = 8192         # tokens
NCORES = 8
NS = N // NCORES     # 1024 tokens per core
DS = D // NCORES     # 512 dout per core

LT = L // 128        # 2 l-tiles
DT = D // 128        # 32 d-tiles
NT = NS // 128       # 8 local n-tiles
HT = DT // 2         # 16 d-tiles per u half

_MAX_WAITS = 1


def _split_waits(nc, mybir, bass_rust):
    """Walrus in this container allows only one sync-wait per instruction;
    move excess waits onto preceding same-engine no-ops."""
    for bb in nc.main_func.blocks:
        new_list = []
        for ins in bb.instructions:
            si = ins.sync_info
            waits = list(si.on_wait) if si is not None else []
            if len(waits) > _MAX_WAITS:
                for i in range(_MAX_WAITS, len(waits), _MAX_WAITS):
                    nop = mybir.InstNoOp(name=f"{ins.name}-wsplit{i}", ins=[], outs=[])
                    nop.engine = ins.engine
                    nop.sync_info = bass_rust.SyncInfo(
                        on_wait=waits[i:i + _MAX_WAITS], on_update=[])
                    new_list.append(nop)
                ins.sync_info = bass_rust.SyncInfo(
                    on_wait=waits[:_MAX_WAITS], on_update=si.on_update)
            new_list.append(ins)
        bb.instructions[:] = new_list


_NC = None


def _build(split_waits=True):
    global _NC
    if _NC is not None and split_waits:
        return _NC
    import bass_rust
    import concourse.bass as bass
    import concourse.mybir as mybir
    import concourse.tile as tile
    from concourse.masks import make_identity
    from contextlib import ExitStack

    f32 = mybir.dt.float32
    bf16 = mybir.dt.bfloat16
    AF = mybir.ActivationFunctionType
    AX = mybir.AxisListType
    ALU = mybir.AluOpType
    RG = [list(range(NCORES))]

    f16 = mybir.dt.float16
    nc = bass.Bass()

    # per-core external I/O
    t_qts = nc.dram_tensor("qTs", [D, L], f16, kind="ExternalInput")
    t_wk = nc.dram_tensor("wk", [D, DS], f16, kind="ExternalInput")
    t_xt = nc.dram_tensor("xT", [D, NS], f16, kind="ExternalInput")
    t_xb = nc.dram_tensor("xb", [NS, D], bf16, kind="ExternalInput")
    t_wvt = nc.dram_tensor("wvT", [D, DS], bf16, kind="ExternalInput")
    t_wvb = nc.dram_tensor("wvb", [1, DS], bf16, kind="ExternalInput")
    t_out = nc.dram_tensor("out", [L, DS], f32, kind="ExternalOutput")

    # collective bounce buffers (input Local, output Shared)
    warm_in = nc.dram_tensor("warm_in", [1, 128], f32)
    warm_out = nc.dram_tensor("warm_out", [NCORES, 128], f32, addr_space="Shared")
    ar_q_in = nc.dram_tensor("ar_q_in", [DS, L], f16)
    ar_q_out = nc.dram_tensor("ar_q_out", [D, L], f16, addr_space="Shared")
    ar_m_in = nc.dram_tensor("ar_m_in", [L, 1], f32)
    ar_m_out = nc.dram_tensor("ar_m_out", [L, 1], f32, addr_space="Shared")
    # u: 32 d-tile blocks + one s' block, single AllReduce payload
    ar_u_in = nc.dram_tensor("ar_u_in", [(DT + 1) * 128, L], bf16)
    ar_u_out = nc.dram_tensor("ar_u_out", [(DT + 1) * 128, L], bf16,
                              addr_space="Shared")

    # inputs are host-permuted to p-major so every DMA chunk reads long
    # contiguous per-partition lines (4-16KB instead of 1KB)
    qts_re = t_qts.ap().rearrange("(p kt) l -> p kt l", p=128)   # [128, 32, 256]
    wk_re = t_wk.ap().rearrange("(p kt) d -> p kt d", p=128)     # [128, 32, 512]
    xt_re = t_xt.ap().rearrange("(p dt) n -> p dt n", p=128)     # [128, 32, 1024]
    xb_re = t_xb.ap().rearrange("(p nt) d -> p nt d", p=128)     # [128, 8, 4096]
    wvt_re = t_wvt.ap().rearrange("(p dt) o -> p dt o", p=128)   # [128, 32, 512]
    # AG-q bounce: p-major WITHIN each core's block, so ship writes and the
    # per-block readback both move 2KB contiguous per-partition lines (the
    # (dt p) layout read 512B lines and cost ~13us before phase 2)
    arq_re = ar_q_in.ap().rearrange("(p dtl) l -> p dtl l", p=128)
    arqo_re = ar_q_out.ap().rearrange("(j p dtl) l -> p j dtl l", p=128, dtl=4)
    aru_re = ar_u_in.ap().rearrange("(t p) l -> p t l", p=128)
    aruo_re = ar_u_out.ap().rearrange("(t p) l -> p t l", p=128)

    with ExitStack() as ctx:
        tc = ctx.enter_context(tile.TileContext(nc))
        const = ctx.enter_context(tc.tile_pool(name="const", bufs=1))
        small = ctx.enter_context(tc.tile_pool(name="small", bufs=1))
        persist = ctx.enter_context(tc.tile_pool(name="persist", bufs=1))

        # ---- warm-up: tiny collective absorbs ncfw cold-start ---------------
        # 8-rank AllGather, same replica group as every other collective (a
        # second communicator halves collective bandwidth - v4 lesson), and
        # same kind as AG-q (an AllReduce-max warm-up made AG-q ~10us slower
        # in two separate experiments, v6/v8).
        nc.gpsimd.collective_compute(
            "AllGather", ALU.bypass, replica_groups=RG,
            ins=[warm_in.ap().opt()], outs=[warm_out.ap().opt()])

        ident_bf = const.tile([128, 128], bf16)
        make_identity(nc, ident_bf[:])
        ident_f = const.tile([128, 128], f32)
        make_identity(nc, ident_f[:])
        ones1 = const.tile([1, 128], f32)
        nc.vector.memset(ones1[:], 1.0)
        bias_sb = const.tile([128, DS], bf16)
        wvb_sb = const.tile([1, DS], bf16)
        nc.scalar.dma_start(wvb_sb[:], t_wvb.ap())
        ones1b = const.tile([1, 128], bf16)
        nc.vector.memset(ones1b[:], 1.0)
        s_blk = const.tile([128, L], bf16)       # s' payload block (zeros + 2 cols)
        nc.vector.memset(s_blk[:], 0.0)
        # prewarm the scalar Exp activation table (1.3us off the softmax path)
        dum = const.tile([1, 2], f32)
        nc.vector.memset(dum[:], 0.0)
        nc.scalar.activation(dum[:], dum[:], AF.Exp)

        # PE clock-gate warm-up during the first input DMAs
        with tc.tile_pool(name="warmps", bufs=1, space="PSUM") as warmps:
            wps = warmps.tile([128, 128], f32)
            for i in range(28):
                nc.tensor.matmul(wps[:], ident_bf[:], ident_bf[:],
                                 start=(i == 0), stop=(i == 27))
            # bias broadcast [1,DS] -> [128,DS] via rank-1 matmul
            bps = warmps.tile([128, DS], f32)
            nc.tensor.matmul(bps[:], ones1b[:], wvb_sb[:], start=True, stop=True)
            nc.vector.tensor_copy(bias_sb[:], bps[:])

        # persistent across phases
        pT = persist.tile([128, NT, L], bf16)        # p.T (0.5MB)
        xb_all = persist.tile([128, NT, D], bf16)    # x_j resident (8MB)

        wv_pool = ctx.enter_context(tc.tile_pool(name="ph4w", bufs=2))
        WVCH = 8                     # d-tiles per wv chunk (1MB)

        def wv_load(c):
            wv_c = wv_pool.tile([128, WVCH, DS], bf16, name="wv_c")
            nc.scalar.dma_start(wv_c[:], wvt_re[:, c * WVCH:(c + 1) * WVCH, :])
            return wv_c

        # ---------------- phase 1: q'T partial = Wk[:,ds_j].T @ q.T ----------
        with tc.tile_pool(name="ph1q", bufs=2) as ph1q, \
             tc.tile_pool(name="ph1wk", bufs=4) as ph1wk, \
             tc.tile_pool(name="ph1ps", bufs=1, space="PSUM") as ph1ps:
            def qts_load(qc):
                qts_c = ph1q.tile([128, 8, L], f16, name="qts_c")
                nc.sync.dma_start(qts_c[:], qts_re[:, qc * 8:(qc + 1) * 8, :])
                return qts_c
            qts_cs = [qts_load(0)]
            qpT_loc = ph1q.tile([128, 4, L], f16, name="qpT_loc")
            ps4 = [ph1ps.tile([128, L], f32, name=f"ph1ps{i}") for i in range(4)]
            KCH = 4                                  # k-tiles per wk chunk (1MB)
            for kc in range(DT // KCH):
                wk_c = ph1wk.tile([128, KCH, DS], f16, name="wk_c")
                nc.sync.dma_start(wk_c[:], wk_re[:, kc * KCH:(kc + 1) * KCH, :])
                if kc % 2 == 0 and kc // 2 + 1 < 4:
                    qts_cs.append(qts_load(kc // 2 + 1))
                for i in range(KCH):
                    kt = kc * KCH + i
                    for dtl in range(4):
                        nc.tensor.matmul(
                            ps4[dtl][:], wk_c[:, i, dtl * 128:(dtl + 1) * 128],
                            qts_cs[kt // 8][:, kt % 8, :],
                            start=(kt == 0), stop=(kt == DT - 1))
            for dtl in range(4):
                nc.vector.tensor_copy(qpT_loc[:, dtl, :], ps4[dtl][:])
            # ship on the sync ring (ahead of the xT stream descriptors)
            nc.sync.dma_start(arq_re, qpT_loc[:])
            nc.gpsimd.collective_compute(
                "AllGather", ALU.bypass, replica_groups=RG,
                ins=[ar_q_in.ap().opt()], outs=[ar_q_out.ap().opt()])

        # q'T readback: one chunk per core block on the scalar ring; phase 2
        # starts as soon as block 0 lands
        qpT = persist.tile([128, DT, L], f16, name="qpT")
        for j in range(NCORES):
            nc.scalar.dma_start(qpT[:, 4 * j:4 * j + 4, :],
                                arqo_re[:, j, :, :])

        # ---------------- phase 2: scores[l, n_j] ----------------------------
        XCH = 8                      # d-tiles per xT chunk (2MB)
        m_both = small.tile([128, 2], f32, name="m_both")
        s_both = small.tile([128, 2], f32, name="s_both")
        p_sb = [persist.tile([128, NS], bf16, name=f'p_sb{i}') for i in range(LT)]
        with tc.tile_pool(name="ph2xt", bufs=3) as xt_pool, \
             tc.tile_pool(name="ph2sc", bufs=1, space="PSUM") as scps_pool:
            score_ps = [[scps_pool.tile([128, 512], f32, name=f'score{i}_{k}')
                         for k in range(2)] for i in range(LT)]
            for c in range(DT // XCH):
                xt_c = xt_pool.tile([128, XCH, NS], f16, name="xt_c")
                nc.sync.dma_start(xt_c[:], xt_re[:, c * XCH:(c + 1) * XCH, :])
                for i in range(XCH):
                    dt = c * XCH + i
                    for lt in range(LT):
                        for nch in range(2):
                            nc.tensor.matmul(
                                score_ps[lt][nch][:],
                                qpT[:, dt, lt * 128:(lt + 1) * 128],
                                xt_c[:, i, nch * 512:(nch + 1) * 512],
                                start=(dt == 0), stop=(dt == DT - 1))

            # xb load rides the sync ring in the post-xT window (v4 loaded it
            # at t=0 on scalar and starved phase 1's wk stream)
            for cb in range(4):
                nc.sync.dma_start(xb_all[:, 2 * cb:2 * cb + 2, :],
                                  xb_re[:, 2 * cb:2 * cb + 2, :])

            # local row max; ship m_j on the (idle) sync ring, then
            # AllReduce-MAX: the CCE computes the global row max in flight.
            negm = small.tile([128, 2], f32, name="negm")
            for lt in range(LT):
                mtmp = small.tile([128, 1], f32, name=f"mtmp{lt}")
                nc.vector.tensor_reduce(mtmp[:], score_ps[lt][0][:], axis=AX.X, op=ALU.max)
                nc.vector.tensor_reduce(m_both[:, lt:lt + 1], score_ps[lt][1][:],
                                        axis=AX.X, op=ALU.max)
                nc.vector.tensor_tensor(m_both[:, lt:lt + 1], m_both[:, lt:lt + 1],
                                        mtmp[:], ALU.max)
                nc.vector.tensor_scalar_mul(negm[:, lt:lt + 1],
                                            m_both[:, lt:lt + 1], -1.0)
            nc.sync.dma_start(
                ar_m_in.ap().rearrange("(lt p) o -> p (lt o)", p=128), m_both[:])
            nc.gpsimd.collective_compute(
                "AllReduce", ALU.max, replica_groups=RG,
                ins=[ar_m_in.ap().opt()], outs=[ar_m_out.ap().opt()])
            for lt in range(LT):
                sp0 = small.tile([128, 1], f32, name=f"sp0_{lt}")
                nc.scalar.activation(p_sb[lt][:, 0:512], score_ps[lt][0][:],
                                     AF.Exp, bias=negm[:, lt:lt + 1], accum_out=sp0[:])
                nc.scalar.activation(p_sb[lt][:, 512:1024], score_ps[lt][1][:],
                                     AF.Exp, bias=negm[:, lt:lt + 1],
                                     accum_out=s_both[:, lt:lt + 1])
                nc.vector.tensor_tensor(s_both[:, lt:lt + 1], s_both[:, lt:lt + 1],
                                        sp0[:], ALU.add)

        # ---------------- transpose p -> pT [n, l] ---------------------------
        with tc.tile_pool(name="tp", bufs=2, space="PSUM") as tpps:
            for lt in range(LT):
                for nt in range(NT):
                    tp = tpps.tile([128, 128], bf16)
                    nc.tensor.transpose(
                        tp[:], p_sb[lt][:, nt * 128:(nt + 1) * 128], ident_bf[:])
                    nc.vector.tensor_copy(pT[:, nt, lt * 128:(lt + 1) * 128], tp[:])

        # ---- factor = exp(m_j - M): readback on scalar ring, math on SCALAR -
        M_sb = small.tile([128, 2], f32, name="M_sb")
        nc.scalar.dma_start(
            M_sb[:], ar_m_out.ap().rearrange("(lt p) o -> p (lt o)", p=128))
        fac2 = small.tile([128, 2], f32, name="fac2")
        for lt in range(LT):
            # fac = exp(-M + m_j)
            nc.scalar.activation(fac2[:, lt:lt + 1], M_sb[:, lt:lt + 1],
                                 AF.Exp, bias=m_both[:, lt:lt + 1], scale=-1.0)
            # s' = s_j * fac (cast to bf16 into the AR payload block)
            nc.scalar.activation(s_blk[:, lt:lt + 1], s_both[:, lt:lt + 1],
                                 AF.Copy, scale=fac2[:, lt:lt + 1])

        u_h = [persist.tile([128, HT, L], bf16, name=f"u_h{h}") for h in range(2)]
        ctx_c = [persist.tile([128, WVCH, L], bf16, name=f"ctx_c{c}")
                 for c in range(4)]
        s_ctx = persist.tile([128, L], bf16, name="s_ctx")
        facb2 = small.tile([128, 2, L], f32, name="facb2")

        with tc.tile_pool(name="ph3ps", bufs=4, space="PSUM") as ph3ps:
            # ---- phase 3 h0: d-tiles 0..15, two per PSUM bank ---------------
            for dp in range(HT // 2):
                psu = ph3ps.tile([128, 2, L], f32)
                for k in range(2):
                    dt = 2 * dp + k
                    for nt in range(NT):
                        nc.tensor.matmul(
                            psu[:, k, :], xb_all[:, nt, dt * 128:(dt + 1) * 128],
                            pT[:, nt, :], start=(nt == 0), stop=(nt == NT - 1))
                nc.vector.tensor_copy(u_h[0][:, 2 * dp:2 * dp + 2, :], psu[:])

            # ---- factor broadcast via PE (tensor queue after h0 MMs; the
            # PSUM->SBUF copies ride the scalar queue) ------------------------
            with tc.tile_pool(name="facps", bufs=1, space="PSUM") as facps_pool:
                fb_ps = facps_pool.tile([128, L], f32, name="fb_ps")
                for lt in range(LT):
                    fac_ps = facps_pool.tile([1, 128], f32, name=f"fac_ps{lt}")
                    nc.tensor.transpose(fac_ps[:], fac2[:, lt:lt + 1], ident_f[:])
                    facr = small.tile([1, 128], f32, name=f"facr{lt}")
                    nc.scalar.activation(facr[:], fac_ps[:], AF.Copy)
                    nc.tensor.matmul(fb_ps[:, lt * 128:(lt + 1) * 128],
                                     ones1[:], facr[:], start=True, stop=True)
                # two copies of the broadcast so rescales handle a whole
                # 2-tile PSUM bank in one vector op
                for k in range(2):
                    nc.scalar.activation(facb2[:, k, :], fb_ps[:], AF.Copy)

            # rescale h0 (paired tiles) and ship its half of the AR payload
            for dp in range(HT // 2):
                nc.vector.tensor_tensor(u_h[0][:, 2 * dp:2 * dp + 2, :],
                                        u_h[0][:, 2 * dp:2 * dp + 2, :],
                                        facb2[:], ALU.mult)
            nc.sync.dma_start(aru_re[:, 0:HT, :], u_h[0][:])
            nc.sync.dma_start(aru_re[:, DT, :], s_blk[:])

            # ---- phase 3 h1: d-tiles 16..31, rescale fused into the cast ----
            for dp in range(HT // 2):
                psu = ph3ps.tile([128, 2, L], f32)
                for k in range(2):
                    dt = HT + 2 * dp + k
                    for nt in range(NT):
                        nc.tensor.matmul(
                            psu[:, k, :], xb_all[:, nt, dt * 128:(dt + 1) * 128],
                            pT[:, nt, :], start=(nt == 0), stop=(nt == NT - 1))
                nc.vector.tensor_tensor(u_h[1][:, 2 * dp:2 * dp + 2, :],
                                        psu[:], facb2[:], ALU.mult)
            nc.sync.dma_start(aru_re[:, HT:DT, :], u_h[1][:])
            nc.gpsimd.collective_compute(
                "AllReduce", ALU.add, replica_groups=RG,
                ins=[ar_u_in.ap().opt()], outs=[ar_u_out.ap().opt()])

        wv_cs = [wv_load(0), wv_load(1), wv_load(2), wv_load(3)]

        # warm bridge across the AR-u wait (no data deps). Sized to span the
        # full AR (~45us at ~150ns/MM) - v5's 150 ended 30us early and
        # phase 4 ran at K=4/8 cold.
        # bridge must span h1-end (~186us) to ctx-readback-done (~252us):
        # ~66us at ~105ns/MM
        with tc.tile_pool(name="warmps2", bufs=1, space="PSUM") as warmps2:
            wps2 = warmps2.tile([128, 128], f32)
            for i in range(640):
                nc.tensor.matmul(wps2[:], ident_bf[:], ident_bf[:],
                                 start=(i == 0), stop=(i == 639))

        # ctx readback (4 chunks, consumed in order by phase 4) + 1/S.
        # Alternate scalar/sync queues so the chunks move in parallel
        # (serial on one queue cost 5.3us between AR-u and phase 4).
        for c in range(4):
            eng = nc.scalar if c % 2 == 0 else nc.sync
            eng.dma_start(ctx_c[c][:],
                          aruo_re[:, c * WVCH:(c + 1) * WVCH, :])
        nc.scalar.dma_start(s_ctx[:], aruo_re[:, DT, :])
        s_f = small.tile([128, 2], f32, name="s_f")
        nc.vector.tensor_copy(s_f[:], s_ctx[:, 0:2])
        rec2 = small.tile([128, 2], f32, name="rec2")
        nc.vector.reciprocal(rec2[:], s_f[:])

        # ---------------- phase 4: out = (ctxT/S).T @ WvT + Wv_b -------------
        with tc.tile_pool(name="ph4ps", bufs=1, space="PSUM") as ph4ps, \
             tc.tile_pool(name="ph4o", bufs=2) as out_pool:
            po = [ph4ps.tile([128, DS], f32, name=f'po{i}') for i in range(LT)]
            for c in range(4):
                for i in range(WVCH):
                    dt = c * WVCH + i
                    for lt in range(LT):
                        nc.tensor.matmul(
                            po[lt][:], ctx_c[c][:, i, lt * 128:(lt + 1) * 128],
                            wv_cs[c][:, i, :], start=(dt == 0), stop=(dt == DT - 1))
            for lt in range(LT):
                o_sb = out_pool.tile([128, DS], f32)
                nc.scalar.activation(o_sb[:], po[lt][:], AF.Copy,
                                     scale=rec2[:, lt:lt + 1])
                nc.vector.tensor_tensor(o_sb[:], o_sb[:], bias_sb[:], ALU.add)
                nc.scalar.dma_start(t_out[lt * 128:(lt + 1) * 128, :], o_sb[:])

    if split_waits:
        _split_waits(nc, mybir, bass_rust)
        _NC = nc
    return nc


last_results = None


def kernel(src_prompts, query, Wk_w, Wk_b, Wv_w, Wv_b):
    global last_results
    import ml_dtypes
    from concourse.bass_utils import run_bass_kernel_spmd

    nc = _build()

    x = np.ascontiguousarray(np.asarray(src_prompts, dtype=np.float32)[0])
    q = np.asarray(query, dtype=np.float32)
    wk = np.asarray(Wk_w, dtype=np.float32)
    wv = np.asarray(Wv_w, dtype=np.float32)
    wvb = np.asarray(Wv_b, dtype=np.float32)
    # Wk_b shifts every score row by a constant -> cancels in softmax.

    def pmajor(a, tiles):
        # [tiles*128, cols] row=t*128+p  ->  row=p*tiles+t (p-major), so each
        # SBUF partition's stream is contiguous in DRAM
        r, c = a.shape
        return np.ascontiguousarray(
            a.reshape(tiles, 128, c).transpose(1, 0, 2).reshape(r, c))

    qT = pmajor(np.ascontiguousarray(q.T).astype(np.float16), DT)
    in_maps = []
    for j in range(NCORES):
        ns, ds = slice(j * NS, (j + 1) * NS), slice(j * DS, (j + 1) * DS)
        xj = x[ns]
        in_maps.append({
            "qTs": qT,
            "wk": pmajor(wk[:, ds].astype(np.float16), DT),
            "xT": pmajor(np.ascontiguousarray(xj.T).astype(np.float16), DT),
            "xb": pmajor(xj.astype(ml_dtypes.bfloat16), NT),
            "wvT": pmajor(np.ascontiguousarray(wv[ds].T).astype(ml_dtypes.bfloat16), DT),
            "wvb": np.ascontiguousarray(wvb[ds][None, :].astype(ml_dtypes.bfloat16)),
        })

    res = run_bass_kernel_spmd(nc, in_maps, core_ids=list(range(NCORES)))
    last_results = res
    out = np.concatenate([res.results[j]["out"] for j in range(NCORES)], axis=1)
    return out[None, :, :]


# revision 43
# speedup vs baseline: 1.0766x; 1.0575x over previous
"""Trainium2 Bass kernel for nn_AttentionProjector (8-core SPMD), v5.

Math: out = softmax(q @ (x@Wk.T).T) @ (x@Wv.T + Wv_b)
Rewritten (FLOP reduction):
    scores = (q @ Wk) @ x.T      (Wk_b cancels in softmax)
    out    = (softmax(scores) @ x) @ Wv.T + Wv_b

Structure (8 cores), v5 notes:
  - ALL collectives use the single 8-rank replica group. (v4 tried a
    pair-group warm-up; the extra communicator halved the bandwidth of
    every later collective: AG-q 18.6->32us, AR halves 30/37us.)
  - warm-up AllGather at t=0 absorbs the ~80us ncfw first-collective
    cold start while phase 1 + input DMAs run.
  - phase 1: q'T slice via Wk[:,ds_j] -> AllGather q'T (ship on sync
    ring ahead of the xT descriptors, doorbell on gpsimd).
  - phase 2: scores = q'T.T @ xT_j (f16). Local row max -> AllReduce-MAX
    (CCE computes the global max in the DMA path; no readback chain).
    m ship + M readback ride HWDGE rings (sync/scalar) - the v4 SWDGE
    path added ~8us latency each way.
  - factor exp(m_j - M) and s' = S_j*f on the SCALAR queue only; facb
    broadcast via PE between h0 and h1 so nothing head-of-line blocks
    the phase-3 PSUM casts (the v3 25us stall).
  - phase 3: uT = x.T @ p.T, xb SBUF-resident, two d-tiles per PSUM
    bank. h0 casts plain + rescale, shipped immediately; h1 casts fuse
    the rescale (psu*facb -> bf16). Single AllReduce for u + s'.
  - phase 4: after a warm bridge, 4 ctx chunks read back + consumed in
    order; out = (ctxT/S).T @ WvT + Wv_b.

Precision: score path f16; values path bf16 (measured 5.5e-3).
"""

import numpy as np

L = 256          # query rows
D = 4096         # d_in == d_out
N = 8192         # tokens
NCORES = 8
NS = N // NCORES     # 1024 tokens per core
DS = D // NCORES     # 512 dout per core

LT = L // 128        # 2 l-tiles
DT = D // 128        # 32 d-tiles
NT = NS // 128       # 8 local n-tiles
HT = DT // 2         # 16 d-tiles per u half

_MAX_WAITS = 1


def _split_waits(nc, mybir, bass_rust):
    """Walrus in this container allows only one sync-wait per instruction;
    move excess waits onto preceding same-engine no-ops."""
    for bb in nc.main_func.blocks:
        new_list = []
        for ins in bb.instructions:
            si = ins.sync_info
            waits = list(si.on_wait) if si is not None else []
            if len(waits) > _MAX_WAITS:
                for i in range(_MAX_WAITS, len(waits), _MAX_WAITS):
                    nop = mybir.InstNoOp(name=f"{ins.name}-wsplit{i}", ins=[], outs=[])
                    nop.engine = ins.engine
                    nop.sync_info = bass_rust.SyncInfo(
                        on_wait=waits[i:i + _MAX_WAITS], on_update=[])
                    new_list.append(nop)
                ins.sync_info = bass_rust.SyncInfo(
                    on_wait=waits[:_MAX_WAITS], on_update=si.on_update)
            new_list.append(ins)
        bb.instructions[:] = new_list


_NC = None


def _build(split_waits=True):
    global _NC
    if _NC is not None and split_waits:
        return _NC
    import bass_rust
    import concourse.bass as bass
    import concourse.mybir as mybir
    import concourse.tile as tile
    from concourse.masks import make_identity
    from contextlib import ExitStack

    f32 = mybir.dt.float32
    bf16 = mybir.dt.bfloat16
    AF = mybir.ActivationFunctionType
    AX = mybir.AxisListType
    ALU = mybir.AluOpType
    RG = [list(range(NCORES))]

    f16 = mybir.dt.float16
    nc = bass.Bass()

    # per-core external I/O
    t_qts = nc.dram_tensor("qTs", [D, L], f16, kind="ExternalInput")
    t_wk = nc.dram_tensor("wk", [D, DS], f16, kind="ExternalInput")
    t_xt = nc.dram_tensor("xT", [D, NS], f16, kind="ExternalInput")
    t_xb = nc.dram_tensor("xb", [NS, D], bf16, kind="ExternalInput")
    t_wvt = nc.dram_tensor("wvT", [D, DS], bf16, kind="ExternalInput")
    t_wvb = nc.dram_tensor("wvb", [1, DS], bf16, kind="ExternalInput")
    t_out = nc.dram_tensor("out", [L, DS], f32, kind="ExternalOutput")

    # collective bounce buffers (input Local, output Shared)
    warm_in = nc.dram_tensor("warm_in", [1, 128], f32)
    warm_out = nc.dram_tensor("warm_out", [NCORES, 128], f32, addr_space="Shared")
    ar_q_in = nc.dram_tensor("ar_q_in", [DS, L], f16)
    ar_q_out = nc.dram_tensor("ar_q_out", [D, L], f16, addr_space="Shared")
    ar_m_in = nc.dram_tensor("ar_m_in", [L, 1], f32)
    ar_m_out = nc.dram_tensor("ar_m_out", [L, 1], f32, addr_space="Shared")
    # u: 32 d-tile blocks + one s' block, single AllReduce payload
    ar_u_in = nc.dram_tensor("ar_u_in", [(DT + 1) * 128, L], bf16)
    ar_u_out = nc.dram_tensor("ar_u_out", [(DT + 1) * 128, L], bf16,
                              addr_space="Shared")

    # inputs are host-permuted to p-major so every DMA chunk reads long
    # contiguous per-partition lines (4-16KB instead of 1KB)
    qts_re = t_qts.ap().rearrange("(p kt) l -> p kt l", p=128)   # [128, 32, 256]
    wk_re = t_wk.ap().rearrange("(p kt) d -> p kt d", p=128)     # [128, 32, 512]
    xt_re = t_xt.ap().rearrange("(p dt) n -> p dt n", p=128)     # [128, 32, 1024]
    xb_re = t_xb.ap().rearrange("(p nt) d -> p nt d", p=128)     # [128, 8, 4096]
    wvt_re = t_wvt.ap().rearrange("(p dt) o -> p dt o", p=128)   # [128, 32, 512]
    # AG-q bounce: p-major WITHIN each core's block, so ship writes and the
    # per-block readback both move 2KB contiguous per-partition lines (the
    # (dt p) layout read 512B lines and cost ~13us before phase 2)
    arq_re = ar_q_in.ap().rearrange("(p dtl) l -> p dtl l", p=128)
    arqo_re = ar_q_out.ap().rearrange("(j p dtl) l -> p j dtl l", p=128, dtl=4)
    aru_re = ar_u_in.ap().rearrange("(t p) l -> p t l", p=128)
    aruo_re = ar_u_out.ap().rearrange("(t p) l -> p t l", p=128)

    with ExitStack() as ctx:
        tc = ctx.enter_context(tile.TileContext(nc))
        const = ctx.enter_context(tc.tile_pool(name="const", bufs=1))
        small = ctx.enter_context(tc.tile_pool(name="small", bufs=1))
        persist = ctx.enter_context(tc.tile_pool(name="persist", bufs=1))

        # ---- warm-up: tiny collective absorbs ncfw cold-start ---------------
        # 8-rank AllGather, same replica group as every other collective (a
        # second communicator halves collective bandwidth - v4 lesson), and
        # same kind as AG-q (an AllReduce-max warm-up made AG-q ~10us slower
        # in two separate experiments, v6/v8).
        nc.gpsimd.collective_compute(
            "AllGather", ALU.bypass, replica_groups=RG,
            ins=[warm_in.ap().opt()], outs=[warm_out.ap().opt()])

        ident_bf = const.tile([128, 128], bf16)
        make_identity(nc, ident_bf[:])
        ident_f = const.tile([128, 128], f32)
        make_identity(nc, ident_f[:])
        ones1 = const.tile([1, 128], f32)
        nc.vector.memset(ones1[:], 1.0)
        bias_sb = const.tile([128, DS], bf16)
        wvb_sb = const.tile([1, DS], bf16)
        nc.scalar.dma_start(wvb_sb[:], t_wvb.ap())
        ones1b = const.tile([1, 128], bf16)
        nc.vector.memset(ones1b[:], 1.0)
        s_blk = const.tile([128, L], bf16)       # s' payload block (zeros + 2 cols)
        nc.vector.memset(s_blk[:], 0.0)
        # prewarm the scalar Exp activation table (1.3us off the softmax path)
        dum = const.tile([1, 2], f32)
        nc.vector.memset(dum[:], 0.0)
        nc.scalar.activation(dum[:], dum[:], AF.Exp)

        # PE clock-gate warm-up during the first input DMAs
        with tc.tile_pool(name="warmps", bufs=1, space="PSUM") as warmps:
            wps = warmps.tile([128, 128], f32)
            for i in range(28):
                nc.tensor.matmul(wps[:], ident_bf[:], ident_bf[:],
                                 start=(i == 0), stop=(i == 27))
            # bias broadcast [1,DS] -> [128,DS] via rank-1 matmul
            bps = warmps.tile([128, DS], f32)
            nc.tensor.matmul(bps[:], ones1b[:], wvb_sb[:], start=True, stop=True)
            nc.vector.tensor_copy(bias_sb[:], bps[:])

        # persistent across phases
        pT = persist.tile([128, NT, L], bf16)        # p.T (0.5MB)
        xb_all = persist.tile([128, NT, D], bf16)    # x_j resident (8MB)

        wv_pool = ctx.enter_context(tc.tile_pool(name="ph4w", bufs=2))
        WVCH = 8                     # d-tiles per wv chunk (1MB)

        def wv_load(c):
            wv_c = wv_pool.tile([128, WVCH, DS], bf16, name="wv_c")
            nc.scalar.dma_start(wv_c[:], wvt_re[:, c * WVCH:(c + 1) * WVCH, :])
            return wv_c

        # ---------------- phase 1: q'T partial = Wk[:,ds_j].T @ q.T ----------
        with tc.tile_pool(name="ph1q", bufs=2) as ph1q, \
             tc.tile_pool(name="ph1wk", bufs=4) as ph1wk, \
             tc.tile_pool(name="ph1ps", bufs=1, space="PSUM") as ph1ps:
            def qts_load(qc):
                qts_c = ph1q.tile([128, 8, L], f16, name="qts_c")
                nc.sync.dma_start(qts_c[:], qts_re[:, qc * 8:(qc + 1) * 8, :])
                return qts_c
            qts_cs = [qts_load(0)]
            qpT_loc = ph1q.tile([128, 4, L], f16, name="qpT_loc")
            ps4 = [ph1ps.tile([128, L], f32, name=f"ph1ps{i}") for i in range(4)]
            KCH = 4                                  # k-tiles per wk chunk (1MB)
            for kc in range(DT // KCH):
                wk_c = ph1wk.tile([128, KCH, DS], f16, name="wk_c")
                nc.sync.dma_start(wk_c[:], wk_re[:, kc * KCH:(kc + 1) * KCH, :])
                if kc % 2 == 0 and kc // 2 + 1 < 4:
                    qts_cs.append(qts_load(kc // 2 + 1))
                for i in range(KCH):
                    kt = kc * KCH + i
                    for dtl in range(4):
                        nc.tensor.matmul(
                            ps4[dtl][:], wk_c[:, i, dtl * 128:(dtl + 1) * 128],
                            qts_cs[kt // 8][:, kt % 8, :],
                            start=(kt == 0), stop=(kt == DT - 1))
            for dtl in range(4):
                nc.vector.tensor_copy(qpT_loc[:, dtl, :], ps4[dtl][:])
            # ship on the sync ring (ahead of the xT stream descriptors)
            nc.sync.dma_start(arq_re, qpT_loc[:])
            nc.gpsimd.collective_compute(
                "AllGather", ALU.bypass, replica_groups=RG,
                ins=[ar_q_in.ap().opt()], outs=[ar_q_out.ap().opt()])

        # q'T readback: one chunk per core block on the scalar ring; phase 2
        # starts as soon as block 0 lands
        qpT = persist.tile([128, DT, L], f16, name="qpT")
        for j in range(NCORES):
            nc.scalar.dma_start(qpT[:, 4 * j:4 * j + 4, :],
                                arqo_re[:, j, :, :])

        # ---------------- phase 2: scores[l, n_j] ----------------------------
        XCH = 8                      # d-tiles per xT chunk (2MB)
        m_both = small.tile([128, 2], f32, name="m_both")
        s_both = small.tile([128, 2], f32, name="s_both")
        p_sb = [persist.tile([128, NS], bf16, name=f'p_sb{i}') for i in range(LT)]
        with tc.tile_pool(name="ph2xt", bufs=3) as xt_pool, \
             tc.tile_pool(name="ph2sc", bufs=1, space="PSUM") as scps_pool:
            score_ps = [[scps_pool.tile([128, 512], f32, name=f'score{i}_{k}')
                         for k in range(2)] for i in range(LT)]
            for c in range(DT // XCH):
                xt_c = xt_pool.tile([128, XCH, NS], f16, name="xt_c")
                nc.sync.dma_start(xt_c[:], xt_re[:, c * XCH:(c + 1) * XCH, :])
                for i in range(XCH):
                    dt = c * XCH + i
                    for lt in range(LT):
                        for nch in range(2):
                            nc.tensor.matmul(
                                score_ps[lt][nch][:],
                                qpT[:, dt, lt * 128:(lt + 1) * 128],
                                xt_c[:, i, nch * 512:(nch + 1) * 512],
                                start=(dt == 0), stop=(dt == DT - 1))

            # xb load rides the sync ring in the post-xT window (v4 loaded it
            # at t=0 on scalar and starved phase 1's wk stream)
            for cb in range(4):
                nc.sync.dma_start(xb_all[:, 2 * cb:2 * cb + 2, :],
                                  xb_re[:, 2 * cb:2 * cb + 2, :])

            # local row max; ship m_j on the (idle) sync ring, then
            # AllReduce-MAX: the CCE computes the global row max in flight.
            negm = small.tile([128, 2], f32, name="negm")
            for lt in range(LT):
                mtmp = small.tile([128, 1], f32, name=f"mtmp{lt}")
                nc.vector.tensor_reduce(mtmp[:], score_ps[lt][0][:], axis=AX.X, op=ALU.max)
                nc.vector.tensor_reduce(m_both[:, lt:lt + 1], score_ps[lt][1][:],
                                        axis=AX.X, op=ALU.max)
                nc.vector.tensor_tensor(m_both[:, lt:lt + 1], m_both[:, lt:lt + 1],
                                        mtmp[:], ALU.max)
            nc.vector.tensor_scalar_mul(negm[:], m_both[:], -1.0)
            nc.sync.dma_start(
                ar_m_in.ap().rearrange("(lt p) o -> p (lt o)", p=128), m_both[:])
            nc.gpsimd.collective_compute(
                "AllReduce", ALU.max, replica_groups=RG,
                ins=[ar_m_in.ap().opt()], outs=[ar_m_out.ap().opt()])
            for lt in range(LT):
                sp0 = small.tile([128, 1], f32, name=f"sp0_{lt}")
                nc.scalar.activation(p_sb[lt][:, 0:512], score_ps[lt][0][:],
                                     AF.Exp, bias=negm[:, lt:lt + 1], accum_out=sp0[:])
                nc.scalar.activation(p_sb[lt][:, 512:1024], score_ps[lt][1][:],
                                     AF.Exp, bias=negm[:, lt:lt + 1],
                                     accum_out=s_both[:, lt:lt + 1])
                nc.vector.tensor_tensor(s_both[:, lt:lt + 1], s_both[:, lt:lt + 1],
                                        sp0[:], ALU.add)

        # ---------------- transpose p -> pT [n, l] ---------------------------
        with tc.tile_pool(name="tp", bufs=2, space="PSUM") as tpps:
            for lt in range(LT):
                for nt in range(NT):
                    tp = tpps.tile([128, 128], bf16)
                    nc.tensor.transpose(
                        tp[:], p_sb[lt][:, nt * 128:(nt + 1) * 128], ident_bf[:])
                    nc.vector.tensor_copy(pT[:, nt, lt * 128:(lt + 1) * 128], tp[:])

        # ---- factor = exp(m_j - M): readback on scalar ring, math on SCALAR -
        M_sb = small.tile([128, 2], f32, name="M_sb")
        nc.scalar.dma_start(
            M_sb[:], ar_m_out.ap().rearrange("(lt p) o -> p (lt o)", p=128))
        fac2 = small.tile([128, 2], f32, name="fac2")
        for lt in range(LT):
            # fac = exp(-M + m_j)
            nc.scalar.activation(fac2[:, lt:lt + 1], M_sb[:, lt:lt + 1],
                                 AF.Exp, bias=m_both[:, lt:lt + 1], scale=-1.0)
            # s' = s_j * fac (cast to bf16 into the AR payload block)
            nc.scalar.activation(s_blk[:, lt:lt + 1], s_both[:, lt:lt + 1],
                                 AF.Copy, scale=fac2[:, lt:lt + 1])

        u_h = [persist.tile([128, HT, L], bf16, name=f"u_h{h}") for h in range(2)]
        ctx_c = [persist.tile([128, WVCH, L], bf16, name=f"ctx_c{c}")
                 for c in range(4)]
        s_ctx = persist.tile([128, L], bf16, name="s_ctx")
        facb2 = small.tile([128, 2, L], f32, name="facb2")

        with tc.tile_pool(name="ph3ps", bufs=4, space="PSUM") as ph3ps:
            # ---- phase 3 h0: d-tiles 0..15, two per PSUM bank ---------------
            for dp in range(HT // 2):
                psu = ph3ps.tile([128, 2, L], f32)
                for k in range(2):
                    dt = 2 * dp + k
                    for nt in range(NT):
                        nc.tensor.matmul(
                            psu[:, k, :], xb_all[:, nt, dt * 128:(dt + 1) * 128],
                            pT[:, nt, :], start=(nt == 0), stop=(nt == NT - 1))
                nc.vector.tensor_copy(u_h[0][:, 2 * dp:2 * dp + 2, :], psu[:])

            # ---- factor broadcast via PE (tensor queue after h0 MMs; the
            # PSUM->SBUF copies ride the scalar queue) ------------------------
            with tc.tile_pool(name="facps", bufs=1, space="PSUM") as facps_pool:
                fb_ps = facps_pool.tile([128, L], f32, name="fb_ps")
                for lt in range(LT):
                    fac_ps = facps_pool.tile([1, 128], f32, name=f"fac_ps{lt}")
                    nc.tensor.transpose(fac_ps[:], fac2[:, lt:lt + 1], ident_f[:])
                    facr = small.tile([1, 128], f32, name=f"facr{lt}")
                    nc.scalar.activation(facr[:], fac_ps[:], AF.Copy)
                    nc.tensor.matmul(fb_ps[:, lt * 128:(lt + 1) * 128],
                                     ones1[:], facr[:], start=True, stop=True)
                # two copies of the broadcast so rescales handle a whole
                # 2-tile PSUM bank in one vector op
                for k in range(2):
                    nc.scalar.activation(facb2[:, k, :], fb_ps[:], AF.Copy)

            # rescale h0 (paired tiles) and ship its half of the AR payload
            for dp in range(HT // 2):
                nc.vector.tensor_tensor(u_h[0][:, 2 * dp:2 * dp + 2, :],
                                        u_h[0][:, 2 * dp:2 * dp + 2, :],
                                        facb2[:], ALU.mult)
            nc.sync.dma_start(aru_re[:, 0:HT, :], u_h[0][:])
            nc.sync.dma_start(aru_re[:, DT, :], s_blk[:])

            # ---- phase 3 h1: d-tiles 16..31, rescale fused into the cast ----
            for dp in range(HT // 2):
                psu = ph3ps.tile([128, 2, L], f32)
                for k in range(2):
                    dt = HT + 2 * dp + k
                    for nt in range(NT):
                        nc.tensor.matmul(
                            psu[:, k, :], xb_all[:, nt, dt * 128:(dt + 1) * 128],
                            pT[:, nt, :], start=(nt == 0), stop=(nt == NT - 1))
                nc.vector.tensor_tensor(u_h[1][:, 2 * dp:2 * dp + 2, :],
                                        psu[:], facb2[:], ALU.mult)
            nc.sync.dma_start(aru_re[:, HT:DT, :], u_h[1][:])
            nc.gpsimd.collective_compute(
                "AllReduce", ALU.add, replica_groups=RG,
                ins=[ar_u_in.ap().opt()], outs=[ar_u_out.ap().opt()])

        wv_cs = [wv_load(0), wv_load(1), wv_load(2), wv_load(3)]

        # warm bridge across the AR-u wait (no data deps). Sized to span the
        # full AR (~45us at ~150ns/MM) - v5's 150 ended 30us early and
        # phase 4 ran at K=4/8 cold.
        # bridge must span h1-end (~186us) to ctx-readback-done (~252us):
        # ~66us at ~105ns/MM
        with tc.tile_pool(name="warmps2", bufs=1, space="PSUM") as warmps2:
            wps2 = warmps2.tile([128, 128], f32)
            for i in range(640):
                nc.tensor.matmul(wps2[:], ident_bf[:], ident_bf[:],
                                 start=(i == 0), stop=(i == 639))

        # ctx readback (4 chunks, consumed in order by phase 4) + 1/S
        for c in range(4):
            nc.scalar.dma_start(ctx_c[c][:],
                                aruo_re[:, c * WVCH:(c + 1) * WVCH, :])
        nc.scalar.dma_start(s_ctx[:], aruo_re[:, DT, :])
        s_f = small.tile([128, 2], f32, name="s_f")
        nc.vector.tensor_copy(s_f[:], s_ctx[:, 0:2])
        rec2 = small.tile([128, 2], f32, name="rec2")
        nc.vector.reciprocal(rec2[:], s_f[:])

        # ---------------- phase 4: out = (ctxT/S).T @ WvT + Wv_b -------------
        with tc.tile_pool(name="ph4ps", bufs=1, space="PSUM") as ph4ps, \
             tc.tile_pool(name="ph4o", bufs=2) as out_pool:
            po = [ph4ps.tile([128, DS], f32, name=f'po{i}') for i in range(LT)]
            for c in range(4):
                for i in range(WVCH):
                    dt = c * WVCH + i
                    for lt in range(LT):
                        nc.tensor.matmul(
                            po[lt][:], ctx_c[c][:, i, lt * 128:(lt + 1) * 128],
                            wv_cs[c][:, i, :], start=(dt == 0), stop=(dt == DT - 1))
            for lt in range(LT):
                o_sb = out_pool.tile([128, DS], f32)
                nc.scalar.activation(o_sb[:], po[lt][:], AF.Copy,
                                     scale=rec2[:, lt:lt + 1])
                nc.vector.tensor_tensor(o_sb[:], o_sb[:], bias_sb[:], ALU.add)
                nc.scalar.dma_start(t_out[lt * 128:(lt + 1) * 128, :], o_sb[:])

    if split_waits:
        _split_waits(nc, mybir, bass_rust)
        _NC = nc
    return nc


last_results = None


def kernel(src_prompts, query, Wk_w, Wk_b, Wv_w, Wv_b):
    global last_results
    import ml_dtypes
    from concourse.bass_utils import run_bass_kernel_spmd

    nc = _build()

    x = np.ascontiguousarray(np.asarray(src_prompts, dtype=np.float32)[0])
    q = np.asarray(query, dtype=np.float32)
    wk = np.asarray(Wk_w, dtype=np.float32)
    wv = np.asarray(Wv_w, dtype=np.float32)
    wvb = np.asarray(Wv_b, dtype=np.float32)
    # Wk_b shifts every score row by a constant -> cancels in softmax.

    def pmajor(a, tiles):
        # [tiles*128, cols] row=t*128+p  ->  row=p*tiles+t (p-major), so each
        # SBUF partition's stream is contiguous in DRAM
        r, c = a.shape
        return np.ascontiguousarray(
            a.reshape(tiles, 128, c).transpose(1, 0, 2).reshape(r, c))

    qT = pmajor(np.ascontiguousarray(q.T).astype(np.float16), DT)
    in_maps = []
    for j in range(NCORES):
        ns, ds = slice(j * NS, (j + 1) * NS), slice(j * DS, (j + 1) * DS)
        xj = x[ns]
        in_maps.append({
            "qTs": qT,
            "wk": pmajor(wk[:, ds].astype(np.float16), DT),
            "xT": pmajor(np.ascontiguousarray(xj.T).astype(np.float16), DT),
            "xb": pmajor(xj.astype(ml_dtypes.bfloat16), NT),
            "wvT": pmajor(np.ascontiguousarray(wv[ds].T).astype(ml_dtypes.bfloat16), DT),
            "wvb": np.ascontiguousarray(wvb[ds][None, :].astype(ml_dtypes.bfloat16)),
        })

    res = run_bass_kernel_spmd(nc, in_maps, core_ids=list(range(NCORES)))
    last_results = res
    out = np.concatenate([res.results[j]["out"] for j in range(NCORES)], axis=1)
    return out[None, :, :]
